# revision 8
# baseline (speedup 1.0000x reference)
"""Causal self-attention TRN2 kernel (8 NeuronCores), v3.

Sharding: 8 cores = (head-group hg in {0,1}) x (batch b in {0..3});
partial y summed pairwise on host.

v3 vs v2: fine-grained software pipelining. qk-projection, v-projection
and output-projection matmul groups are emitted as generators that yield
every few matmuls; the attention jj-loop pumps one chunk per slot so the
PE queue always has independent work interleaved between the
ACT-dependent scores/PV chain. Attention for i-block 0 starts right
after its own q/k features (fi 0,4) and v tiles are done; the remaining
qk features stream in as fillers.
"""

import collections

import numpy as np
import ml_dtypes

import concourse.bacc as bacc
import concourse.mybir as mybir
import concourse.tile as tile
from concourse.bass_utils import run_bass_kernel_spmd

F32 = mybir.dt.float32
BF16 = mybir.dt.bfloat16
EXP = mybir.ActivationFunctionType.Exp

B, T, C = 4, 2048, 1024
NH, HD = 16, 64
HPC = 8                      # heads per core
FH = HPC * HD                # 512: per-core q/k/v feature width
NCORES = 8
LAG = 2                      # scores->PV software-pipeline depth (j-tiles)

_CACHE = {}


def build_nc():
    nc = bacc.Bacc()
    xT_d = nc.dram_tensor("xT", [C, T], BF16, kind="ExternalInput")
    wqkvT_d = nc.dram_tensor("wqkvT", [C, 3 * FH], BF16, kind="ExternalInput")
    wprojT_d = nc.dram_tensor("wprojT", [FH, C], BF16, kind="ExternalInput")
    y_d = nc.dram_tensor("y", [T, C], F32, kind="ExternalOutput")

    NKT = C // 128           # 8 c-tiles (contraction for qkv)
    NTT = T // 128           # 16 t-tiles
    NTC = T // 512           # 4 t-chunks / i-blocks

    with tile.TileContext(nc) as tc:
        with (
            tc.tile_pool(name="qkt", bufs=1) as qkt_pool,
            tc.tile_pool(name="vp", bufs=1) as v_pool,
            tc.tile_pool(name="wts", bufs=1) as w_pool,
            tc.tile_pool(name="attnt", bufs=1) as attnt_pool,
            tc.tile_pool(name="xc", bufs=1) as x_pool,
            tc.tile_pool(name="pt", bufs=1) as pt_pool,
            tc.tile_pool(name="stage", bufs=1) as stage_pool,
            tc.tile_pool(name="nrm", bufs=1) as nrm_pool,
            tc.tile_pool(name="psS", bufs=1, space="PSUM") as psS,
            tc.tile_pool(name="psPV", bufs=1, space="PSUM") as psPV,
            tc.tile_pool(name="psM", bufs=1, space="PSUM") as psM,
        ):
            qkT = [qkt_pool.tile([128, T], BF16, tag=f"qkt{i}", name=f"qkt{i}")
                   for i in range(8)]
            v_sb = [v_pool.tile([128, HPC * 65], BF16, tag=f"v{i}", name=f"v{i}")
                    for i in range(NTT)]
            wqk = w_pool.tile([128, NKT * 2 * FH], BF16, tag="wqk", name="wqk")
            wv = w_pool.tile([128, NKT * FH], BF16, tag="wv", name="wv")
            wprojT = w_pool.tile([128, 4 * C], BF16, tag="wp", name="wp")
            attnT = [attnt_pool.tile([128, T], BF16, tag=f"at{g}",
                                     name=f"at{g}") for g in range(4)]
            # master causal mask: mask[j, c] = 1.0 if c >= j + 384 else 0.
            # Slice [384-d : 384-d+w] masks a straddle whose invalid region
            # starts d columns left of the mul range.
            mask = w_pool.tile([128, 512], BF16, tag="mask", name="mask")
            nc.vector.memset(mask[:], 1.0)
            nc.gpsimd.affine_select(
                out=mask[:], in_=mask[:],
                compare_op=mybir.AluOpType.is_ge, fill=0.0,
                base=-384, pattern=[[1, 512]], channel_multiplier=-1)

            # prewarm the ACT exp table
            warm = w_pool.tile([1, 8], F32, tag="warm", name="warm")
            nc.vector.memset(warm[:], 0.0)
            nc.scalar.activation(warm[0:1, :], warm[0:1, :], EXP, scale=1.0)

            # ---- weight DMAs (first slices early; bulk on gpsimd queue) ----
            nc.sync.dma_start(
                out=wqk[:].rearrange("p (k f) -> p k f", k=NKT)[:, 0:2, :],
                in_=wqkvT_d[0:256, 0:2 * FH].rearrange("(k p) f -> p k f",
                                                       p=128))
            nc.sync.dma_start(
                out=wqk[:].rearrange("p (k f) -> p k f", k=NKT)[:, 2:NKT, :],
                in_=wqkvT_d[256:C, 0:2 * FH].rearrange("(k p) f -> p k f",
                                                       p=128))
            nc.gpsimd.dma_start(
                out=wv[:].rearrange("p (k f) -> p k f", k=NKT),
                in_=wqkvT_d[0:C, 2 * FH:3 * FH].rearrange("(k p) f -> p k f",
                                                          p=128))
            nc.gpsimd.dma_start(
                out=wprojT[:].rearrange("p (g f) -> p g f", g=4),
                in_=wprojT_d[0:FH, :].rearrange("(g p) f -> p g f", p=128))

            xcs = {}

            def emit_xdma(tcb):
                xc = x_pool.tile([128, NKT * 512], BF16, tag="xc", bufs=2,
                                 name=f"xc{tcb}")
                if tcb == 0:
                    nc.sync.dma_start(
                        out=xc[:].rearrange("p (k t) -> p k t",
                                            k=NKT)[:, 0:2, :],
                        in_=xT_d[0:256, 0:512].rearrange(
                            "(k p) t -> p k t", p=128))
                    nc.sync.dma_start(
                        out=xc[:].rearrange("p (k t) -> p k t",
                                            k=NKT)[:, 2:NKT, :],
                        in_=xT_d[256:C, 0:512].rearrange(
                            "(k p) t -> p k t", p=128))
                else:
                    nc.sync.dma_start(
                        out=xc[:].rearrange("p (k t) -> p k t", k=NKT),
                        in_=xT_d[0:C, tcb * 512:(tcb + 1) * 512].rearrange(
                            "(k p) t -> p k t", p=128))
                xcs[tcb] = xc

            # ---------------- filler generators ----------------
            def gen_qk_fi(tcb, fi):
                xc = xcs[tcb]
                ps = psM.tile([128, 512], F32, tag="mm512", bufs=2,
                              name=f"psqk{tcb}_{fi}")
                for k in range(NKT):
                    nc.tensor.matmul(
                        ps[:],
                        wqk[:, k * 1024 + fi * 128:k * 1024 + (fi + 1) * 128],
                        xc[:, k * 512:(k + 1) * 512],
                        start=(k == 0), stop=(k == NKT - 1))
                    if k == 3:
                        yield
                nc.vector.tensor_copy(
                    out=qkT[fi][:, tcb * 512:(tcb + 1) * 512], in_=ps[:])
                yield

            def gen_v(tcb, tl):
                xc = xcs[tcb]
                ti = tcb * 4 + tl
                ps = psM.tile([128, 512], F32, tag="mm512", bufs=2,
                              name=f"psv{ti}")
                for k in range(NKT):
                    nc.tensor.matmul(
                        ps[:],
                        xc[:, k * 512 + tl * 128:k * 512 + (tl + 1) * 128],
                        wv[:, k * FH:(k + 1) * FH],
                        start=(k == 0), stop=(k == NKT - 1))
                    if k == 3:
                        yield
                vt = v_sb[ti]
                vv = vt[:].rearrange("p (h x) -> p h x", h=HPC)
                nc.vector.memset(vt[:], 1.0)
                nc.vector.tensor_copy(
                    out=vv[:, :, 0:64],
                    in_=ps[:].rearrange("p (h x) -> p h x", h=HPC))
                yield

            def gen_proj(bi):
                for ti in range(4 * bi, 4 * bi + 4):
                    for fc in range(2):
                        po = psM.tile([128, 512], F32, tag="mm512", bufs=2,
                                      name=f"po{ti}_{fc}")
                        for g in range(4):
                            nc.tensor.matmul(
                                po[:],
                                attnT[g][:, ti * 128:(ti + 1) * 128],
                                wprojT[:, g * C + fc * 512:
                                       g * C + (fc + 1) * 512],
                                start=(g == 0), stop=(g == 3))
                        ot = stage_pool.tile([128, 512], F32, tag="ot",
                                             bufs=2, name=f"ot{ti}_{fc}")
                        nc.vector.tensor_copy(out=ot[:], in_=po[:])
                        nc.sync.dma_start(
                            out=y_d[ti * 128:(ti + 1) * 128,
                                    fc * 512:(fc + 1) * 512],
                            in_=ot[:])
                        yield

            fillers = collections.deque()

            def pump(n=1):
                for _ in range(n):
                    while fillers:
                        try:
                            next(fillers[0])
                            return
                        except StopIteration:
                            fillers.popleft()
                    return

            def drain():
                while fillers:
                    pump()

            # ---------------- attention ----------------
            def attention(bi):
                njt = 4 * bi + 4
                for hp in range(4):
                    qt = qkT[hp]
                    kt = qkT[4 + hp]
                    pts = []
                    pvs = [psPV.tile([65, 512], F32, tag=f"pv{par}", bufs=1,
                                     name=f"pv{bi}_{hp}_{par}")
                           for par in range(2)]

                    def emit_scores(jj, bi=bi, hp=hp, qt=qt, kt=kt, pts=pts):
                        r0 = jj - 4 * bi
                        lo = 128 * r0 if (r0 > 0 and bi > 0) else 0
                        sps = psS.tile([128, 1024], F32, tag="sps", bufs=2,
                                       name=f"sps{bi}_{hp}_{jj}")
                        sv = sps[:].rearrange("p (b i) -> p b i", b=2)
                        for par in range(2):
                            off = par * 64
                            nc.tensor.matmul(
                                sv[:, par, lo:512],
                                kt[off:off + 64, jj * 128:(jj + 1) * 128],
                                qt[off:off + 64,
                                   bi * 512 + lo:(bi + 1) * 512],
                                start=True, stop=True)
                        pt = pt_pool.tile([128, 1024], BF16, tag="pt",
                                          bufs=8, name=f"pt{bi}_{hp}_{jj}")
                        tv = pt[:].rearrange("p (b i) -> p b i", b=2)
                        nc.scalar.activation(tv[:, :, lo:512],
                                             sv[:, :, lo:512], EXP,
                                             scale=0.125)
                        if r0 >= 0:
                            d0 = 128 * r0
                            for par in range(2):
                                nc.vector.tensor_mul(
                                    out=tv[:, par, lo:d0 + 128],
                                    in0=tv[:, par, lo:d0 + 128],
                                    in1=mask[:, 384 - (d0 - lo):512])
                        pts.append((pt, lo))

                    def emit_pv(jj, stop, bi=bi, hp=hp, pts=pts, pvs=pvs):
                        pt, lo = pts[jj]
                        ptv = pt[:].rearrange("p (b i) -> p b i", b=2)
                        for par in range(2):
                            h = 2 * hp + par
                            nc.tensor.matmul(
                                pvs[par][:, lo:512],
                                v_sb[jj][:, h * 65:h * 65 + 65],
                                ptv[:, par, lo:512],
                                start=(jj == 0), stop=stop,
                                skip_group_check=(lo > 0))

                    # full-range (off-diag) closer for trimmed groups
                    if bi > 0:
                        cl = 4 * bi - 1
                        pv_order = (list(range(cl))
                                    + list(range(4 * bi, njt)) + [cl])
                    else:
                        pv_order = list(range(njt))

                    for jj in range(njt):
                        emit_scores(jj)
                        if jj >= LAG:
                            emit_pv(pv_order[jj - LAG], jj - LAG == njt - 1)
                        pump()
                    pump(2)
                    for p in range(max(0, njt - LAG), njt):
                        emit_pv(pv_order[p], p == njt - 1)

                    for par in range(2):
                        pv = pvs[par]
                        rec = nrm_pool.tile([1, 512], F32, tag="rec",
                                            bufs=2, name=f"rec{bi}_{hp}_{par}")
                        nc.vector.reciprocal_approx_fast(
                            out=rec[0:1, :], in_=pv[64:65, :])
                        bc = nrm_pool.tile([64, 512], F32, tag="bc",
                                           bufs=2, name=f"bc{bi}_{hp}_{par}")
                        nc.gpsimd.partition_broadcast(bc[:, :], rec[0:1, :])
                        nc.vector.tensor_mul(
                            out=attnT[hp][par * 64:par * 64 + 64,
                                          bi * 512:(bi + 1) * 512],
                            in0=pv[0:64, :], in1=bc[:, :])
                    pump(2)

            # ---------------- schedule ----------------
            emit_xdma(0)
            emit_xdma(1)
            # head: only fi 0,4 (head-pair 0) and v(0) before attention(0)
            for step in gen_qk_fi(0, 0):
                pass
            for step in gen_qk_fi(0, 4):
                pass
            for tl in range(4):
                for step in gen_v(0, tl):
                    pass
            for fi in (1, 5, 2, 6, 3, 7):
                fillers.append(gen_qk_fi(0, fi))

            for tcb in range(NTC):
                if tcb + 2 < NTC:
                    emit_xdma(tcb + 2)
                # queue next chunk's qk/v: pumped during attention(tcb),
                # drained before attention(tcb+1) needs them
                if tcb + 1 < NTC:
                    for fi in (0, 4, 1, 5, 2, 6, 3, 7):
                        fillers.append(gen_qk_fi(tcb + 1, fi))
                    for tl in range(4):
                        fillers.append(gen_v(tcb + 1, tl))
                attention(tcb)
                drain()
                fillers.append(gen_proj(tcb))
            drain()
    nc.compile()
    return nc


def _get_nc():
    if "nc" not in _CACHE:
        _CACHE["nc"] = build_nc()
    return _CACHE["nc"]


def kernel(x, w_qkv, w_proj, _trace=False):
    x = np.asarray(x, dtype=np.float32)
    w_qkv = np.asarray(w_qkv, dtype=np.float32)
    w_proj = np.asarray(w_proj, dtype=np.float32)
    bf = ml_dtypes.bfloat16

    nc = _get_nc()
    in_maps = []
    for c in range(NCORES):
        hg, b = c // 4, c % 4
        xT = np.ascontiguousarray(x[b].T).astype(bf)            # [1024, 2048]
        rows = []
        for sec in range(3):                                     # q, k, v
            rows.append(w_qkv[sec * C + hg * FH: sec * C + (hg + 1) * FH])
        wqkvT = np.ascontiguousarray(
            np.concatenate(rows, 0).T).astype(bf)                # [1024, 1536]
        wprojT = np.ascontiguousarray(
            w_proj[:, hg * FH:(hg + 1) * FH].T).astype(bf)
        in_maps.append({"xT": xT, "wqkvT": wqkvT, "wprojT": wprojT})

    res = run_bass_kernel_spmd(nc, in_maps, list(range(NCORES)), trace=_trace)
    if _trace:
        _CACHE["exec_time_ns"] = res.exec_time_ns

    y = np.empty((B, T, C), dtype=np.float32)
    for b in range(B):
        y[b] = res.results[b]["y"] + res.results[4 + b]["y"]
    return y


# revision 14
# speedup vs baseline: 1.0035x; 1.0035x over previous
"""Causal self-attention TRN2 kernel (8 NeuronCores), v3.

Sharding: 8 cores = (head-group hg in {0,1}) x (batch b in {0..3});
partial y summed pairwise on host.

v3 vs v2: fine-grained software pipelining. qk-projection, v-projection
and output-projection matmul groups are emitted as generators that yield
every few matmuls; the attention jj-loop pumps one chunk per slot so the
PE queue always has independent work interleaved between the
ACT-dependent scores/PV chain. Attention for i-block 0 starts right
after its own q/k features (fi 0,4) and v tiles are done; the remaining
qk features stream in as fillers.
"""

import collections

import numpy as np
import ml_dtypes

import concourse.bacc as bacc
import concourse.mybir as mybir
import concourse.tile as tile
from concourse.bass_utils import run_bass_kernel_spmd

F32 = mybir.dt.float32
BF16 = mybir.dt.bfloat16
EXP = mybir.ActivationFunctionType.Exp

B, T, C = 4, 2048, 1024
NH, HD = 16, 64
HPC = 8                      # heads per core
FH = HPC * HD                # 512: per-core q/k/v feature width
NCORES = 8
LAG = 3                      # scores->PV software-pipeline depth (j-tiles)

_CACHE = {}


def build_nc():
    nc = bacc.Bacc()
    # pre-tiled on host: every DMA is per-partition contiguous
    xT_d = nc.dram_tensor("xT", [T // 512, 128, 8 * 512], BF16,
                          kind="ExternalInput")
    wqk_d = nc.dram_tensor("wqk", [128, 8 * 1024], BF16, kind="ExternalInput")
    wv_d = nc.dram_tensor("wv", [128, 8 * 512], BF16, kind="ExternalInput")
    wproj_d = nc.dram_tensor("wproj", [128, 4 * 1024], BF16,
                             kind="ExternalInput")
    y_d = nc.dram_tensor("y", [T, C], F32, kind="ExternalOutput")

    NKT = C // 128           # 8 c-tiles (contraction for qkv)
    NTT = T // 128           # 16 t-tiles
    NTC = T // 512           # 4 t-chunks / i-blocks

    with tile.TileContext(nc) as tc:
        with (
            tc.tile_pool(name="qkt", bufs=1) as qkt_pool,
            tc.tile_pool(name="vp", bufs=1) as v_pool,
            tc.tile_pool(name="wts", bufs=1) as w_pool,
            tc.tile_pool(name="attnt", bufs=1) as attnt_pool,
            tc.tile_pool(name="xc", bufs=1) as x_pool,
            tc.tile_pool(name="pt", bufs=1) as pt_pool,
            tc.tile_pool(name="stage", bufs=1) as stage_pool,
            tc.tile_pool(name="nrm", bufs=1) as nrm_pool,
            tc.tile_pool(name="psS", bufs=1, space="PSUM") as psS,
            tc.tile_pool(name="psPV", bufs=1, space="PSUM") as psPV,
            tc.tile_pool(name="psM", bufs=1, space="PSUM") as psM,
        ):
            qkT = [qkt_pool.tile([128, T], BF16, tag=f"qkt{i}", name=f"qkt{i}")
                   for i in range(8)]
            v_sb = [v_pool.tile([128, HPC * 65], BF16, tag=f"v{i}", name=f"v{i}")
                    for i in range(NTT)]
            wqk = w_pool.tile([128, NKT * 2 * FH], BF16, tag="wqk", name="wqk")
            wv = w_pool.tile([128, NKT * FH], BF16, tag="wv", name="wv")
            wprojT = w_pool.tile([128, 4 * C], BF16, tag="wp", name="wp")
            attnT = [attnt_pool.tile([128, T], BF16, tag=f"at{g}",
                                     name=f"at{g}") for g in range(4)]
            # master causal mask: mask[j, c] = 1.0 if c >= j + 384 else 0.
            # Slice [384-d : 384-d+w] masks a straddle whose invalid region
            # starts d columns left of the mul range.
            mask = w_pool.tile([128, 512], BF16, tag="mask", name="mask")
            nc.vector.memset(mask[:], 1.0)
            nc.gpsimd.affine_select(
                out=mask[:], in_=mask[:],
                compare_op=mybir.AluOpType.is_ge, fill=0.0,
                base=-384, pattern=[[1, 512]], channel_multiplier=-1)

            # prewarm the ACT exp table
            warm = w_pool.tile([1, 8], F32, tag="warm", name="warm")
            nc.vector.memset(warm[:], 0.0)
            nc.scalar.activation(warm[0:1, :], warm[0:1, :], EXP, scale=1.0)

            # ---- input DMAs spread across engine queues (parallel DGE
            # rings): scalar+sync are HWDGE; wqk/xc0 split so the first
            # qk matmuls can start after ~0.5MB instead of 4MB ----
            nc.scalar.dma_start(out=wqk[:, 0:2048], in_=wqk_d[:, 0:2048])
            nc.scalar.dma_start(out=wqk[:, 2048:NKT * 1024],
                                in_=wqk_d[:, 2048:NKT * 1024])
            nc.gpsimd.dma_start(out=wv[:], in_=wv_d[:, :])
            xcs = {}

            def emit_xdma(tcb, eng=None):
                xc = x_pool.tile([128, NKT * 512], BF16, tag="xc", bufs=2,
                                 name=f"xc{tcb}")
                eng = eng or nc.sync
                if tcb == 0:
                    nc.sync.dma_start(out=xc[:, 0:1024], in_=xT_d[0, :, 0:1024])
                    nc.sync.dma_start(out=xc[:, 1024:NKT * 512],
                                      in_=xT_d[0, :, 1024:NKT * 512])
                else:
                    eng.dma_start(out=xc[:], in_=xT_d[tcb, :, :])
                xcs[tcb] = xc

            # ---------------- filler generators ----------------
            def gen_qk_fi(tcb, fi):
                xc = xcs[tcb]
                ps = psM.tile([128, 512], F32, tag="mm512", bufs=2,
                              name=f"psqk{tcb}_{fi}")
                for k in range(NKT):
                    nc.tensor.matmul(
                        ps[:],
                        wqk[:, k * 1024 + fi * 128:k * 1024 + (fi + 1) * 128],
                        xc[:, k * 512:(k + 1) * 512],
                        start=(k == 0), stop=(k == NKT - 1))
                    if k == 3:
                        yield
                nc.vector.tensor_copy(
                    out=qkT[fi][:, tcb * 512:(tcb + 1) * 512], in_=ps[:])
                yield

            def gen_v(tcb, tl):
                xc = xcs[tcb]
                ti = tcb * 4 + tl
                ps = psM.tile([128, 512], F32, tag="mm512", bufs=2,
                              name=f"psv{ti}")
                for k in range(NKT):
                    nc.tensor.matmul(
                        ps[:],
                        xc[:, k * 512 + tl * 128:k * 512 + (tl + 1) * 128],
                        wv[:, k * FH:(k + 1) * FH],
                        start=(k == 0), stop=(k == NKT - 1))
                    if k == 3:
                        yield
                vt = v_sb[ti]
                vv = vt[:].rearrange("p (h x) -> p h x", h=HPC)
                nc.vector.memset(vt[:], 1.0)
                nc.vector.tensor_copy(
                    out=vv[:, :, 0:64],
                    in_=ps[:].rearrange("p (h x) -> p h x", h=HPC))
                yield

            def gen_proj(bi):
                for ti in range(4 * bi, 4 * bi + 4):
                    for fc in range(2):
                        po = psM.tile([128, 512], F32, tag="mm512", bufs=2,
                                      name=f"po{ti}_{fc}")
                        for g in range(4):
                            nc.tensor.matmul(
                                po[:],
                                attnT[g][:, ti * 128:(ti + 1) * 128],
                                wprojT[:, g * C + fc * 512:
                                       g * C + (fc + 1) * 512],
                                start=(g == 0), stop=(g == 3))
                        ot = stage_pool.tile([128, 512], F32, tag="ot",
                                             bufs=2, name=f"ot{ti}_{fc}")
                        nc.vector.tensor_copy(out=ot[:], in_=po[:])
                        nc.sync.dma_start(
                            out=y_d[ti * 128:(ti + 1) * 128,
                                    fc * 512:(fc + 1) * 512],
                            in_=ot[:])
                        yield

            fillers = collections.deque()

            def pump(n=1):
                for _ in range(n):
                    while fillers:
                        try:
                            next(fillers[0])
                            return
                        except StopIteration:
                            fillers.popleft()
                    return

            def drain():
                while fillers:
                    pump()

            # ---------------- attention ----------------
            def attention(bi):
                njt = 4 * bi + 4
                for hp in range(4):
                    qt = qkT[hp]
                    kt = qkT[4 + hp]
                    pts = []
                    pvs = [psPV.tile([65, 512], F32, tag=f"pv{par}", bufs=1,
                                     name=f"pv{bi}_{hp}_{par}")
                           for par in range(2)]

                    def emit_scores(jj, bi=bi, hp=hp, qt=qt, kt=kt, pts=pts):
                        r0 = jj - 4 * bi
                        lo = 128 * r0 if (r0 > 0 and bi > 0) else 0
                        sps = psS.tile([128, 1024], F32, tag="sps", bufs=2,
                                       name=f"sps{bi}_{hp}_{jj}")
                        sv = sps[:].rearrange("p (b i) -> p b i", b=2)
                        for par in range(2):
                            off = par * 64
                            nc.tensor.matmul(
                                sv[:, par, lo:512],
                                kt[off:off + 64, jj * 128:(jj + 1) * 128],
                                qt[off:off + 64,
                                   bi * 512 + lo:(bi + 1) * 512],
                                start=True, stop=True)
                        pt = pt_pool.tile([128, 1024], BF16, tag="pt",
                                          bufs=8, name=f"pt{bi}_{hp}_{jj}")
                        tv = pt[:].rearrange("p (b i) -> p b i", b=2)
                        nc.scalar.activation(tv[:, :, lo:512],
                                             sv[:, :, lo:512], EXP,
                                             scale=0.125)
                        if r0 >= 0:
                            d0 = 128 * r0
                            for par in range(2):
                                nc.vector.tensor_mul(
                                    out=tv[:, par, lo:d0 + 128],
                                    in0=tv[:, par, lo:d0 + 128],
                                    in1=mask[:, 384 - (d0 - lo):512])
                        pts.append((pt, lo))

                    def emit_pv(jj, stop, bi=bi, hp=hp, pts=pts, pvs=pvs):
                        pt, lo = pts[jj]
                        ptv = pt[:].rearrange("p (b i) -> p b i", b=2)
                        for par in range(2):
                            h = 2 * hp + par
                            nc.tensor.matmul(
                                pvs[par][:, lo:512],
                                v_sb[jj][:, h * 65:h * 65 + 65],
                                ptv[:, par, lo:512],
                                start=(jj == 0), stop=stop,
                                skip_group_check=(lo > 0))

                    # full-range (off-diag) closer for trimmed groups
                    if bi > 0:
                        cl = 4 * bi - 1
                        pv_order = (list(range(cl))
                                    + list(range(4 * bi, njt)) + [cl])
                    else:
                        pv_order = list(range(njt))

                    for jj in range(njt):
                        emit_scores(jj)
                        if jj >= LAG:
                            emit_pv(pv_order[jj - LAG], jj - LAG == njt - 1)
                        pump()
                    for p in range(max(0, njt - LAG), njt):
                        emit_pv(pv_order[p], p == njt - 1)
                    # filler work for the PE while the normalize chain
                    # drains psPV (next head-pair's PV waits on it)
                    pump(3)

                    for par in range(2):
                        pv = pvs[par]
                        rec = nrm_pool.tile([1, 512], F32, tag="rec",
                                            bufs=2, name=f"rec{bi}_{hp}_{par}")
                        nc.vector.reciprocal_approx_fast(
                            out=rec[0:1, :], in_=pv[64:65, :])
                        bc = nrm_pool.tile([64, 512], F32, tag="bc",
                                           bufs=2, name=f"bc{bi}_{hp}_{par}")
                        nc.gpsimd.partition_broadcast(bc[:, :], rec[0:1, :])
                        nc.vector.tensor_mul(
                            out=attnT[hp][par * 64:par * 64 + 64,
                                          bi * 512:(bi + 1) * 512],
                            in0=pv[0:64, :], in1=bc[:, :])
                    pump(2)

            # ---------------- schedule ----------------
            emit_xdma(0)
            emit_xdma(1, nc.gpsimd)
            nc.gpsimd.dma_start(out=wprojT[:], in_=wproj_d[:, :])
            # head: only fi 0,4 (head-pair 0) and v(0) before attention(0)
            for step in gen_qk_fi(0, 0):
                pass
            for step in gen_qk_fi(0, 4):
                pass
            for tl in range(4):
                for step in gen_v(0, tl):
                    pass
            for fi in (1, 5, 2, 6, 3, 7):
                fillers.append(gen_qk_fi(0, fi))

            for tcb in range(NTC):
                if tcb + 2 < NTC:
                    emit_xdma(tcb + 2, nc.gpsimd)
                # queue next chunk's qk/v: pumped during attention(tcb),
                # drained before attention(tcb+1) needs them
                if tcb + 1 < NTC:
                    for fi in (0, 4, 1, 5, 2, 6, 3, 7):
                        fillers.append(gen_qk_fi(tcb + 1, fi))
                    for tl in range(4):
                        fillers.append(gen_v(tcb + 1, tl))
                attention(tcb)
                drain()
                fillers.append(gen_proj(tcb))
            drain()
    nc.compile()
    return nc


def _get_nc():
    if "nc" not in _CACHE:
        _CACHE["nc"] = build_nc()
    return _CACHE["nc"]


def kernel(x, w_qkv, w_proj, _trace=False):
    x = np.asarray(x, dtype=np.float32)
    w_qkv = np.asarray(w_qkv, dtype=np.float32)
    w_proj = np.asarray(w_proj, dtype=np.float32)
    bf = ml_dtypes.bfloat16

    nc = _get_nc()
    in_maps = []
    for c in range(NCORES):
        hg, b = c // 4, c % 4
        # x tiled: [tcb, p, k*512+t] with element (k*128+p, tcb*512+t)
        xt = np.ascontiguousarray(
            x[b].T.reshape(8, 128, 4, 512).transpose(2, 1, 0, 3)
            .reshape(4, 128, 4096)).astype(bf)
        rows = []
        for sec in range(3):                                     # q, k, v
            rows.append(w_qkv[sec * C + hg * FH: sec * C + (hg + 1) * FH])
        wqkvT = np.concatenate(rows, 0).T                        # [1024, 1536]
        wqk = np.ascontiguousarray(
            wqkvT[:, 0:1024].reshape(8, 128, 1024).transpose(1, 0, 2)
            .reshape(128, 8192)).astype(bf)
        wv = np.ascontiguousarray(
            wqkvT[:, 1024:1536].reshape(8, 128, 512).transpose(1, 0, 2)
            .reshape(128, 4096)).astype(bf)
        wproj = np.ascontiguousarray(
            w_proj[:, hg * FH:(hg + 1) * FH].T.reshape(4, 128, 1024)
            .transpose(1, 0, 2).reshape(128, 4096)).astype(bf)
        in_maps.append({"xT": xt, "wqk": wqk, "wv": wv, "wproj": wproj})

    res = run_bass_kernel_spmd(nc, in_maps, list(range(NCORES)), trace=_trace)
    if _trace:
        _CACHE["exec_time_ns"] = res.exec_time_ns

    y = np.empty((B, T, C), dtype=np.float32)
    for b in range(B):
        y[b] = res.results[b]["y"] + res.results[4 + b]["y"]
    return y
